# revision 5
# baseline (speedup 1.0000x reference)
"""LoRA-MoE grouped conv2d on 8 TRN2 NeuronCores (Bass/Tile).

Strategy (data-parallel over batch, 4 samples/core):
  out[b] = conv2d(x[b], weight + SCALING*delta[argmax(scores[b])], pad=1)

Host prep (cheap layout/reshape only):
  - argmax routing, gather per-sample LoRA factors
  - weightT: base weight transposed to matmul-lhsT layout [cin, tap, cout]
  - AtapT/BhatT: lora_A/lora_B rearranged so the per-sample delta weight in
    lhsT layout is a single [36]x[128,256] matmul per (tap, cin-chunk)

Device (per core, per sample):
  - delta matmuls (18x [36K,128M,256N]) + DVE add onto base weightT
  - x DMA'd into a zero-padded [cin, 58, 58] SBUF image
  - conv as 9 shifted matmuls x 2 cin chunks accumulated in PSUM
    ([128K,128M,448N] per (cout-chunk, 8-row block)), fp32r dtype
  - PSUM -> SBUF copy -> DMA out
"""

import numpy as np

import concourse.bass as bass
import concourse.mybir as mybir
import concourse.tile as tile_mod
from concourse.tile import TileContext
from concourse.vector_clock import ScopedClock
from concourse.bass_utils import run_bass_kernel_spmd

B, E, CIN, COUT, K, H, W = 32, 5, 256, 256, 3, 56, 56
R = 4
SCALING = 16.0 / R
N_CORES = 8
BPC = B // N_CORES          # samples per core
HP, WP = H + 2, W + 2       # padded image
NROW = 8                    # output rows per PSUM tile
NCHUNK = NROW * W           # 448 free elements per matmul
F32 = mybir.dt.float32
F32R = mybir.dt.float32r

# Walrus in this container rejects multi-wait CTRL instructions ("Too many
# sync wait commands" on the Tile tail Drain). Re-emit the tail with the
# global-clock waits split across single-wait NOPs on the SP queue.
_orig_drain_and_barrier = tile_mod.TileContext._drain_and_barrier


def _patched_drain_and_barrier(self, tick_clock, wait_clock):
    gc = tick_clock.global_clock
    for proc in range(len(gc)):
        tick = gc[proc]
        if tick <= 0:
            continue
        nop = self.nc.sync.nop(nofuse=True)
        sc = ScopedClock()
        sc.require_at_least(None, proc, tick)
        wait_clock.add_sem_waits(nop.ins, sc)
    self.nc.sync.drain()
    self.nc.all_engine_barrier()
    popped = self.nc._tile_sem_poison_stack.pop()
    assert popped is self._sem_poison
    self.nc.clear_and_free_semaphores(list(self.sems.allocated().values()))
    self.nc.all_engine_barrier()


tile_mod.TileContext._drain_and_barrier = _patched_drain_and_barrier

# The same 1-wait limit applies to every CoreV3 instruction encoding (LW,
# CTRL, ...). Rewrite the BIR JSON just before walrus: any instruction
# carrying N>1 sem waits gets N-1 single-wait NoOps inserted immediately
# before it on the same engine (program order per engine = block order).
import orjson as _orjson
import concourse.bass2jax as _bass2jax
from concourse.bass_utils import compile_bir_kernel as _orig_compile_bir_kernel


def _split_bir_waits(bir_json: bytes) -> bytes:
    d = _orjson.loads(bir_json)
    changed = False
    for fn in d.get("functions", []):
        for bl in fn.get("blocks", []):
            insts = bl.get("instructions", [])
            out = []
            for inst in insts:
                si = inst.get("sync_info") or {}
                waits = si.get("on_wait") or []
                if len(waits) > 1:
                    changed = True
                    for k, w in enumerate(waits[:-1]):
                        out.append(
                            {
                                "debug": inst.get("debug", 0),
                                "engine": inst["engine"],
                                "ins": [],
                                "outs": [],
                                "name": f"{inst['name']}-wsplit{k}",
                                "opcode": "NoOp",
                                "sync_info": {"on_update": [], "on_wait": [w]},
                            }
                        )
                    si["on_wait"] = [waits[-1]]
                out.append(inst)
            bl["instructions"] = out
    return _orjson.dumps(d) if changed else bir_json


def _patched_compile_bir_kernel(bir_json, tmpdir, neff_name="file.neff"):
    return _orig_compile_bir_kernel(_split_bir_waits(bir_json), tmpdir, neff_name=neff_name)


_bass2jax.compile_bir_kernel = _patched_compile_bir_kernel


def build_nc():
    nc = bass.Bass()
    x_in = nc.declare_dram_parameter("x", [BPC, CIN, H, W], F32, isOutput=False)
    wt_in = nc.declare_dram_parameter("weightT", [2, 128, 9, COUT], F32, isOutput=False)
    at_in = nc.declare_dram_parameter("atapt", [36, BPC, 9, COUT], F32, isOutput=False)
    bt_in = nc.declare_dram_parameter("bhatt", [36, BPC, COUT], F32, isOutput=False)
    out = nc.declare_dram_parameter("out", [BPC, COUT, H, W], F32, isOutput=True)

    with TileContext(nc) as tc:
        with (
            tc.tile_pool(name="const", bufs=1) as cpool,
            tc.tile_pool(name="xp", bufs=2) as xpool,
            tc.tile_pool(name="wtp", bufs=2) as wtpool,
            tc.tile_pool(name="op", bufs=4) as opool,
            tc.tile_pool(name="dps", bufs=2, space="PSUM") as dpsum,
            tc.tile_pool(name="cps", bufs=4, space="PSUM") as cpsum,
        ):
            wT = cpool.tile([128, 2, 9, COUT], F32, tag="wT")
            for c in range(2):
                nc.sync.dma_start(out=wT[:, c], in_=wt_in[c])
            at = cpool.tile([36, BPC, 9, COUT], F32R, tag="at")
            nc.gpsimd.dma_start(out=at[:], in_=at_in[:])
            bt = cpool.tile([36, BPC, COUT], F32R, tag="bt")
            nc.gpsimd.dma_start(out=bt[:], in_=bt_in[:])

            for b in range(BPC):
                # ---- padded input image [128, cin-chunk, 58, 58] ----
                xp = xpool.tile([128, 2, HP, WP], F32R, tag="xp")
                for c in range(2):
                    nc.gpsimd.memset(xp[:, c].bitcast(F32), 0.0)
                    nc.gpsimd.dma_start(
                        out=xp[:, c, 1 : HP - 1, 1 : WP - 1],
                        in_=x_in[b, c * 128 : (c + 1) * 128],
                    )

                # ---- fused per-sample weights Wt = weightT + delta ----
                wt = wtpool.tile([128, 2, 9, COUT], F32R, tag="wt")
                for c in range(2):
                    for t in range(9):
                        dps = dpsum.tile([128, COUT], F32, tag="dps")
                        nc.tensor.matmul(
                            out=dps[:],
                            lhsT=at[:, b, t, c * 128 : (c + 1) * 128],
                            rhs=bt[:, b],
                            start=True,
                            stop=True,
                        )
                        nc.vector.tensor_add(
                            out=wt[:, c, t], in0=wT[:, c, t], in1=dps[:]
                        )

                # ---- conv: 2 cout chunks x 7 row-blocks, 18-matmul PSUM groups
                for o in range(2):
                    for hc in range(H // NROW):
                        h0 = hc * NROW
                        cps = cpsum.tile([128, NROW, W], F32, tag="cps")
                        n = 0
                        for c in range(2):
                            for t in range(9):
                                kh, kw = t // 3, t % 3
                                nc.tensor.matmul(
                                    out=cps[:],
                                    lhsT=wt[
                                        :, c, t, o * 128 : (o + 1) * 128
                                    ],
                                    rhs=xp[
                                        :, c, h0 + kh : h0 + kh + NROW, kw : kw + W
                                    ],
                                    start=(n == 0),
                                    stop=(n == 17),
                                )
                                n += 1
                        ot = opool.tile([128, NROW, W], F32, tag="ot")
                        nc.any.tensor_copy(out=ot[:], in_=cps[:])
                        nc.sync.dma_start(
                            out=out[b, o * 128 : (o + 1) * 128, h0 : h0 + NROW],
                            in_=ot[:],
                        )
    return nc


def _host_prep(x, scores, weight, lora_A, lora_B):
    experts = np.argmax(scores, axis=1)  # [B]
    # base weight in lhsT layout: [cin-chunk, cin128, tap, cout]
    weightT = np.ascontiguousarray(
        weight.transpose(1, 2, 3, 0).reshape(2, 128, 9, COUT)
    ).astype(np.float32)
    # AtapT[e,t][j*12+r, i] = SCALING * lora_A[e][r, i*9+t-768j], j=(i*9+t)//768
    iv = np.arange(CIN)
    AtapT = np.zeros((E, 9, 36, CIN), np.float32)
    for t in range(9):
        j = (iv * 9 + t) // (CIN * K)
        col = (iv * 9 + t) - (CIN * K) * j
        for e in range(E):
            for r in range(R * K):
                AtapT[e, t, j * 12 + r, iv] = lora_A[e, r, col] * SCALING
    # BhatT[e][j*12+r, o] = lora_B[e][3o+j, r]
    BhatT = np.ascontiguousarray(
        lora_B.reshape(E, COUT, K, R * K).transpose(0, 2, 3, 1).reshape(E, 36, COUT)
    ).astype(np.float32)
    return experts, weightT, AtapT, BhatT


_CACHE = {}


def kernel(x, scores, weight, lora_A, lora_B):
    x = np.asarray(x, np.float32)
    scores = np.asarray(scores, np.float32)
    weight = np.asarray(weight, np.float32)
    lora_A = np.asarray(lora_A, np.float32)
    lora_B = np.asarray(lora_B, np.float32)

    experts, weightT, AtapT, BhatT = _host_prep(x, scores, weight, lora_A, lora_B)

    in_maps = []
    for core in range(N_CORES):
        sl = slice(core * BPC, (core + 1) * BPC)
        ex = experts[sl]
        # [BPC,9,36,*] -> [36, BPC, 9, *] so each SBUF partition is contiguous
        atapt = np.ascontiguousarray(AtapT[ex].transpose(2, 0, 1, 3))
        bhatt = np.ascontiguousarray(BhatT[ex].transpose(1, 0, 2))
        in_maps.append(
            {
                "x": np.ascontiguousarray(x[sl]),
                "weightT": weightT,
                "atapt": atapt,
                "bhatt": bhatt,
            }
        )

    if "nc" not in _CACHE:
        _CACHE["nc"] = build_nc()
    res = run_bass_kernel_spmd(_CACHE["nc"], in_maps, list(range(N_CORES)))
    out = np.concatenate([res.results[c]["out"] for c in range(N_CORES)], axis=0)
    return out
